# revision 1
# baseline (speedup 1.0000x reference)
"""Trainium2 Bass kernel for the LTC (liquid time-constant) memory cell.

Model (see reference): v-state recurrence over T=128 timesteps, each with 6
ODE unfold iterations:
    v' = (cm_t*v + gl*vl + num_syn) / (cm_t + gl + den_syn + eps)
with 2 recurrent synapses per neuron (self: u, pair: (u+dim)%U) and one
sensory synapse (source d = u%dim).

Sharding: 8 cores; core c owns the 128 neuron *pairs* {u=c*128+p,
u+1024} for p in [0,128), with the FULL batch B=32. Each partition p holds
one pair, so every per-neuron parameter is a per-partition scalar [128,1]:
 - sigmoid arguments fuse into one ACT instruction each
   (activation(func=Sigmoid, scale=sigma, bias=-mu*sigma)),
 - synapse combines fuse into scalar_tensor_tensor ((s*w)+acc),
 - sensory NS/DS fuse into tensor_scalar double-op ((sig*a)+b).
Both halves of a pair live on the same core, so the pair-synapse source is
a local tile — no cross-core traffic in the time loop.

Input x is preloaded to SBUF once (core c only needs x[:, :, c*128:(c+1)*128],
since pair (u, u+1024) shares the sensory source d = c*128+p). The input
affine (input_w/input_b) and sensory-mu fold into the ACT scale/bias on the
host; the output affine is applied on the host after gathering.
"""

import numpy as np

import concourse.bacc as bacc
import concourse.mybir as mybir
from concourse import tile
from concourse.tile_rust import add_dep_helper
from concourse.bass_utils import run_bass_kernel_spmd

ODE_UNFOLDS = 6
EPS = 1e-8
B = 32
T = 128
DIM = 1024
U = 2 * DIM
NCORES = 8
P = 128  # partitions = pairs per core

F32 = mybir.dt.float32
AF = mybir.ActivationFunctionType
OP = mybir.AluOpType

# pp column indices (per half; half B adds NPARAM)
# State is carried as w = v + 1 so that w' = (num+den)/den; biases,
# GG and the num-weights are pre-adjusted for the shift.
(C_SIG0, C_B0P, C_SIG1, C_B1P, C_W0, C_W1, C_W0E, C_W1E,
 C_CMT, C_GLV, C_GCME, C_SSIG, C_NSMS, C_SPSW, C_WES,
 C_WPS, C_GGP) = range(17)
NPARAM = 17


def _softplus(x):
    x = x.astype(np.float64)
    return np.log1p(np.exp(-np.abs(x))) + np.maximum(x, 0.0)


def _build_nc(fused_erev=True, G=1, wbufs=4):
    """G: batch split into G independent pipelined groups (FD = B//G)."""
    BG = B // G
    nc = bacc.Bacc(trn_type="TRN2")
    xin_d = nc.dram_tensor("xin", [P, T * B], F32, kind="ExternalInput")
    pp_d = nc.dram_tensor("pp", [P, 2 * NPARAM], F32, kind="ExternalInput")
    out_d = nc.dram_tensor("out", [P, B], F32, kind="ExternalOutput")

    with tile.TileContext(nc) as tc:
        with tc.tile_pool(name="const", bufs=1) as cpool, \
             tc.tile_pool(name="work", bufs=wbufs) as wpool:
            xin = cpool.tile([P, T * B], F32, tag="xin", name="xin_t")
            pp = cpool.tile([P, 2 * NPARAM], F32, tag="pp", name="pp_t")
            nc.sync.dma_start(xin[:], xin_d[:])
            nc.sync.dma_start(pp[:], pp_d[:])

            def par(h, c):  # per-partition scalar AP for half h param c
                j = h * NPARAM + c
                return pp[:, j:j + 1]

            # state tiles (w = v + 1), per (half, group), ping-pong
            v = [[[cpool.tile([P, BG], F32, tag=f"v{h}{g}{i}",
                              name=f"v{h}{g}{i}") for i in range(2)]
                  for g in range(G)] for h in range(2)]
            for h in range(2):
                for g in range(G):
                    nc.vector.memset(v[h][g][0][:], 1.0)

            def wtile(tag):
                return wpool.tile([P, BG], F32, tag=tag, name=tag)

            def sens_sig(t, h, g):
                xt = xin[:, t * B + g * BG: t * B + (g + 1) * BG]
                sg = wtile(f"sg{h}{g}")
                nc.scalar.activation(
                    sg[:], xt, AF.Sigmoid,
                    bias=par(h, C_NSMS), scale=par(h, C_SSIG))
                return sg

            def sens_ds(sg, h, g):
                d_t = wtile(f"ds{h}{g}")
                nc.vector.tensor_scalar(
                    d_t[:], sg[:], par(h, C_SPSW), par(h, C_GCME),
                    OP.mult, OP.add)
                return d_t

            def sens_nd(sg, h, g):
                n_t = wtile(f"nd{h}{g}")
                nc.vector.tensor_scalar(
                    n_t[:], sg[:], par(h, C_WPS), par(h, C_GGP),
                    OP.mult, OP.add)
                return n_t

            def sig(h, slot, g, vin, scol, bcol):
                s = wtile(f"s{slot}{h}{g}")
                bi = nc.scalar.activation(s[:], vin[:], AF.Sigmoid,
                                          bias=par(h, bcol),
                                          scale=par(h, scol))
                return s, bi

            # per-group rolling state
    
            cur = [0] * G
            sg_c = [[sens_sig(0, h, g) for g in range(G)] for h in range(2)]
            ds = [[sens_ds(sg_c[h][g], h, g) for g in range(G)]
                  for h in range(2)]
            nd = [[sens_nd(sg_c[h][g], h, g) for g in range(G)]
                  for h in range(2)]
            s0A = [None] * G
            s1A = [None] * G
            s0B = [None] * G
            s1B = [None] * G
            for g in range(G):
                s0A[g], _ = sig(0, 0, g, v[0][g][0], C_SIG0, C_B0P)
                s1A[g], _ = sig(0, 1, g, v[1][g][0], C_SIG1, C_B1P)
                s0B[g], _ = sig(1, 0, g, v[1][g][0], C_SIG0, C_B0P)
                s1B[g], _ = sig(1, 1, g, v[0][g][0], C_SIG1, C_B1P)
            sg_n = [[None] * G for _ in range(2)]
            ds_n = [[None] * G for _ in range(2)]
            nd_n = [[None] * G for _ in range(2)]
            for t in range(T):
                more = t + 1 < T
                for k in range(ODE_UNFOLDS):
                    # ---- window ops (deps from previous unfold) ----
                    qA = [wtile(f"qA{g}") for g in range(G)]
                    qB = [wtile(f"qB{g}") for g in range(G)]
                    d1A = [wtile(f"d1A{g}") for g in range(G)]
                    d1B = [wtile(f"d1B{g}") for g in range(G)]
                    for g in range(G):
                        nc.vector.scalar_tensor_tensor(
                            qA[g][:], v[0][g][cur[g]][:], par(0, C_CMT),
                            nd[0][g][:], OP.mult, OP.add)
                        nc.vector.scalar_tensor_tensor(
                            qB[g][:], v[1][g][cur[g]][:], par(1, C_CMT),
                            nd[1][g][:], OP.mult, OP.add)
                        nc.vector.scalar_tensor_tensor(
                            d1A[g][:], s0A[g][:], par(0, C_W0),
                            ds[0][g][:], OP.mult, OP.add)
                        nc.vector.scalar_tensor_tensor(
                            d1B[g][:], s1B[g][:], par(1, C_W1),
                            ds[1][g][:], OP.mult, OP.add)
                    if not fused_erev:
                        m1A = [wtile(f"m1A{g}") for g in range(G)]
                        m1B = [wtile(f"m1B{g}") for g in range(G)]
                        for g in range(G):
                            nc.vector.scalar_tensor_tensor(
                                m1A[g][:], s0A[g][:], par(0, C_W0E),
                                qA[g][:], OP.mult, OP.add)
                            nc.vector.scalar_tensor_tensor(
                                m1B[g][:], s1B[g][:], par(1, C_W1E),
                                qB[g][:], OP.mult, OP.add)
                    # sensory fillers for next timestep
                    if more and k == 4:
                        for g in range(G):
                            ds_n[0][g] = sens_ds(sg_n[0][g], 0, g)
                            nd_n[0][g] = sens_nd(sg_n[0][g], 0, g)
                    if more and k == 5:
                        for g in range(G):
                            ds_n[1][g] = sens_ds(sg_n[1][g], 1, g)
                            nd_n[1][g] = sens_nd(sg_n[1][g], 1, g)
                    # ---- tails, interleaved across halves and groups ----
                    denA = [wtile(f"denA{g}") for g in range(G)]
                    denB = [wtile(f"denB{g}") for g in range(G)]
                    rA = [wtile(f"rA{g}") for g in range(G)]
                    rB = [wtile(f"rB{g}") for g in range(G)]
                    for g in range(G):
                        nc.vector.scalar_tensor_tensor(
                            denA[g][:], s1A[g][:], par(0, C_W1),
                            d1A[g][:], OP.mult, OP.add)
                        nc.vector.scalar_tensor_tensor(
                            denB[g][:], s0B[g][:], par(1, C_W0),
                            d1B[g][:], OP.mult, OP.add)
                    for g in range(G):
                        nc.vector.reciprocal(rA[g][:], denA[g][:])
                        nc.vector.reciprocal(rB[g][:], denB[g][:])
                    if fused_erev:
                        mA, mB = qA, qB
                    else:
                        mA = [wtile(f"mA{g}") for g in range(G)]
                        mB = [wtile(f"mB{g}") for g in range(G)]
                        for g in range(G):
                            nc.vector.scalar_tensor_tensor(
                                mA[g][:], s1A[g][:], par(0, C_W1E),
                                m1A[g][:], OP.mult, OP.add)
                            nc.vector.scalar_tensor_tensor(
                                mB[g][:], s0B[g][:], par(1, C_W0E),
                                m1B[g][:], OP.mult, OP.add)
                    for g in range(G):
                        nxt = 1 - cur[g]
                        nc.vector.tensor_mul(v[0][g][nxt][:], mA[g][:],
                                             rA[g][:])
                        n_s0A, _ = sig(0, 0, g, v[0][g][nxt], C_SIG0, C_B0P)
                        nc.vector.tensor_mul(v[1][g][nxt][:], mB[g][:],
                                             rB[g][:])
                        n_s1A, _ = sig(0, 1, g, v[1][g][nxt], C_SIG1, C_B1P)
                        n_s0B, bi_s0B = sig(1, 0, g, v[1][g][nxt],
                                            C_SIG0, C_B0P)
                        n_s1B, bi_s1B = sig(1, 1, g, v[0][g][nxt],
                                            C_SIG1, C_B1P)
                        add_dep_helper(bi_s0B.ins, bi_s1B.ins, sync=True,
                                       reason="s1B off the critical ACT slot")
                        s0A[g], s1A[g] = n_s0A, n_s1A
                        s0B[g], s1B[g] = n_s0B, n_s1B
                        cur[g] = nxt
                    # mid-timestep sensory sigmoids (ACT slack)
                    if more and k == 2:
                        for g in range(G):
                            sg_n[0][g] = sens_sig(t + 1, 0, g)
                    if more and k == 3:
                        for g in range(G):
                            sg_n[1][g] = sens_sig(t + 1, 1, g)
                if more:
                    for h in range(2):
                        for g in range(G):
                            ds[h][g] = ds_n[h][g]
                            nd[h][g] = nd_n[h][g]

            for g in range(G):
                nc.sync.dma_start(
                    out_d[:, g * BG:(g + 1) * BG], v[0][g][cur[g]][:])
    nc.compile()
    return nc


_NC_CACHE = {}


def _get_nc(fused_erev=True):
    if fused_erev not in _NC_CACHE:
        _NC_CACHE[fused_erev] = _build_nc(fused_erev)
    return _NC_CACHE[fused_erev]


def _host_params(c, gleak, vleak, cm, w, sigma, mu, erev,
                 sens_w, sens_sigma, sens_mu, sens_erev,
                 input_w, input_b):
    """pp tensor [128, 2*NPARAM] for core c."""
    d = c * P + np.arange(P)
    pp = np.zeros((P, 2 * NPARAM), np.float32)
    for h in range(2):
        u = h * DIM + d
        sp_w = _softplus(w[u])                       # [P,2]
        sp_gl = _softplus(gleak[u])
        cmt = _softplus(cm[u]) * ODE_UNFOLDS
        o = h * NPARAM
        # state shift w = v + 1: sigmoid biases absorb -sigma, GG absorbs
        # -cmt (so q = cmt*w + ND == cmt*v + NS + DS).
        pp[:, o + C_SIG0] = sigma[u, 0]
        pp[:, o + C_B0P] = -(mu[u, 0] + 1.0) * sigma[u, 0]
        pp[:, o + C_SIG1] = sigma[u, 1]
        pp[:, o + C_B1P] = -(mu[u, 1] + 1.0) * sigma[u, 1]
        pp[:, o + C_W0] = sp_w[:, 0]
        pp[:, o + C_W1] = sp_w[:, 1]
        pp[:, o + C_W0E] = sp_w[:, 0] * (1.0 + erev[u, 0])
        pp[:, o + C_W1E] = sp_w[:, 1] * (1.0 + erev[u, 1])
        pp[:, o + C_CMT] = cmt
        pp[:, o + C_GLV] = sp_gl * vleak[u]
        pp[:, o + C_GCME] = cmt + sp_gl + EPS
        pp[:, o + C_SSIG] = sens_sigma[u] * input_w[d]
        pp[:, o + C_NSMS] = (input_b[d] - sens_mu[u]) * sens_sigma[u]
        pp[:, o + C_SPSW] = _softplus(sens_w[u])
        pp[:, o + C_WES] = _softplus(sens_w[u]) * sens_erev[u]
        pp[:, o + C_WPS] = pp[:, o + C_SPSW] + pp[:, o + C_WES]
        pp[:, o + C_GGP] = pp[:, o + C_GCME] + pp[:, o + C_GLV] - cmt
    return pp


def kernel(inputs, gleak, vleak, cm, w, sigma, mu, erev,
           sens_w, sens_sigma, sens_mu, sens_erev,
           input_w, input_b, output_w, output_b, _trace=False):
    inputs = np.asarray(inputs, np.float32)
    args = dict(gleak=np.asarray(gleak, np.float32),
                vleak=np.asarray(vleak, np.float32),
                cm=np.asarray(cm, np.float32),
                w=np.asarray(w, np.float32),
                sigma=np.asarray(sigma, np.float32),
                mu=np.asarray(mu, np.float32),
                erev=np.asarray(erev, np.float32),
                sens_w=np.asarray(sens_w, np.float32),
                sens_sigma=np.asarray(sens_sigma, np.float32),
                sens_mu=np.asarray(sens_mu, np.float32),
                sens_erev=np.asarray(sens_erev, np.float32),
                input_w=np.asarray(input_w, np.float32),
                input_b=np.asarray(input_b, np.float32))

    in_maps = []
    for c in range(NCORES):
        xc = inputs[:, :, c * P:(c + 1) * P]          # [B,T,P]
        xin = np.ascontiguousarray(
            xc.transpose(2, 1, 0).reshape(P, T * B))  # [P, t*B+b]
        in_maps.append({"xin": xin, "pp": _host_params(c, **args)})

    fused = bool(np.allclose(args["erev"], -1.0))
    nc = _get_nc(fused)
    res = run_bass_kernel_spmd(nc, in_maps, core_ids=list(range(NCORES)),
                               trace=_trace)

    out = np.zeros((B, DIM), np.float32)
    for c in range(NCORES):
        out[:, c * P:(c + 1) * P] = res.results[c]["out"].T
    out = out - 1.0  # state was carried as w = v + 1
    out = out * np.asarray(output_w, np.float32) + np.asarray(output_b, np.float32)
    if _trace:
        kernel.last_results = res
    return out

